# revision 4
# baseline (speedup 1.0000x reference)
"""CRF Viterbi decoder (MLP 768->384->9 + 11-state CRF) on 8 Trainium2 cores.

Strategy (data-parallel over batch, zero cross-core communication):
- Each core gets 8 of the 64 batches. MLP runs as PE matmuls (fp32) with
  tokens in the free dim; tanh/bias on the scalar engine.
- Logits are PE-transposed to token-major, bounced through DRAM, and
  reloaded batch-major [8, T*9] for the scan.
- The Viterbi scan is reduced to the 9 real states (start/end states can
  never win for t>=1; vit0 makes step 0 closed-form), run sequentially on
  the vector engine with exactly the reference's fp32 op ordering:
      vt0 = W_t[b,p] + Tr[n,p];  R_t = max_p vt0;  W_{t+1} = R_t + logit_t
- Backpointers are NOT computed in the scan; they are recomputed afterwards
  in one batched pass over all t on 128 partitions via
  eq = (vt0 == R) ; ptr~ = min_p(eq ? (p-16) : 0)   (first-max semantics).
- Backtrack: one fused scalar_tensor_tensor per step:
      onehot = (iota-16 == idx~) * ptr~_t ; idx~' = sum(onehot)
- preds = shifted indices + 16, cast to int32.
"""

import sys

if "/opt/trn_rl_repo" not in sys.path:
    sys.path.insert(0, "/opt/trn_rl_repo")

import numpy as np

B, T, D, HID, L9 = 64, 512, 768, 384, 9
NCORES = 8
BL = B // NCORES          # 8 batches per core
KT1 = D // 128            # 6 k-tiles, layer 1
MT1 = HID // 128          # 3 m-tiles, layer 1
NCH = (BL * T) // 512     # 8 token chunks of 512
SHIFT = 16.0              # index shift so sel values are < 0

_cache = {}


def _np_fallback(inputs, labels_mask, W1, b1, W2, b2, transition):
    """Pure-numpy reference (only used if labels_mask is not all ones)."""
    Bq, Tq, Dq = inputs.shape
    h = np.tanh(inputs.reshape(-1, Dq) @ W1.T + b1)
    logits = (h @ W2.T + b2).reshape(Bq, Tq, -1)
    NEG = -10000.0
    L = logits.shape[-1] + 2
    logits = np.concatenate(
        [logits, np.full((Bq, Tq, 2), NEG, np.float32)], -1)
    start, end = L - 2, L - 1
    lens = labels_mask.sum(-1)
    vit = np.full((Bq, L), NEG, np.float32)
    vit[:, start] = 0.0
    c = lens.astype(np.int64).copy()
    ptrs = np.zeros((Tq, Bq, L), np.int32)
    for t in range(Tq):
        vt = vit[:, None, :] + transition[None, :, :]
        ptrs[t] = vt.argmax(-1)
        vit_n = vt.max(-1) + logits[:, t]
        active = (c > 0)[:, None]
        vit_new = np.where(active, vit_n, vit)
        last = (c == 1)[:, None].astype(np.float32)
        vit = vit_new + last * transition[end][None, :]
        c -= 1
    idx = vit.argmax(-1).astype(np.int32)
    ys = np.zeros((Tq, Bq), np.int32)
    cur = idx
    for t in range(Tq - 1, -1, -1):
        cur = ptrs[t, np.arange(Bq), cur]
        ys[t] = cur
    preds = np.concatenate([ys[1:], idx[None]], 0).T
    return preds.astype(np.int32)


def _insert_bcast(ap, pos, count):
    from concourse import bass
    dims = list(ap.ap)
    dims.insert(pos, [0, count])
    return bass.AP(ap.tensor, ap.offset, dims)


def _build_program():
    from concourse import bass, bacc, tile, mybir

    f32 = mybir.dt.float32
    i32 = mybir.dt.int32
    ADD = mybir.AluOpType.add
    MULT = mybir.AluOpType.mult
    MAX = mybir.AluOpType.max
    MIN = mybir.AluOpType.min
    EQ = mybir.AluOpType.is_equal
    AX = mybir.AxisListType.X
    ACT = mybir.ActivationFunctionType

    nc = bacc.Bacc("TRN2", target_bir_lowering=False, debug=False,
                   num_devices=NCORES)

    xt_d = nc.dram_tensor("xt", [D, BL * T], f32, kind="ExternalInput")
    w1t_d = nc.dram_tensor("w1t", [128, KT1 * HID], f32, kind="ExternalInput")
    w2t_d = nc.dram_tensor("w2t", [128, MT1 * L9], f32, kind="ExternalInput")
    b1_d = nc.dram_tensor("b1c", [128, MT1], f32, kind="ExternalInput")
    b2_d = nc.dram_tensor("b2c", [L9, 1], f32, kind="ExternalInput")
    trflat_d = nc.dram_tensor("trflat", [128, 81], f32, kind="ExternalInput")
    iota81_d = nc.dram_tensor("iota81", [128, 81], f32, kind="ExternalInput")
    trcol9_d = nc.dram_tensor("trcol9", [BL, L9], f32, kind="ExternalInput")
    trow10_d = nc.dram_tensor("trow10", [BL, L9], f32, kind="ExternalInput")
    iota9_d = nc.dram_tensor("iota9", [BL, L9], f32, kind="ExternalInput")
    ident_d = nc.dram_tensor("ident", [L9, L9], f32, kind="ExternalInput")
    preds_d = nc.dram_tensor("preds", [BL, T], i32, kind="ExternalOutput")

    with tile.TileContext(nc) as tc:
        with (
            tc.tile_pool(name="const", bufs=1) as cpool,
            tc.tile_pool(name="xt", bufs=2) as xpool,
            tc.tile_pool(name="h", bufs=2) as hpool,
            tc.tile_pool(name="lg", bufs=2) as lgpool,
            tc.tile_pool(name="scan", bufs=1) as spool,
            tc.tile_pool(name="mm", bufs=2, space="PSUM") as psum_mm,
            tc.tile_pool(name="lgp", bufs=2, space="PSUM") as psum_lg,
            tc.tile_pool(name="tp", bufs=2, space="PSUM") as psum_tp,
            tc.tile_pool(name="dram", bufs=1, space="DRAM") as dpool,
        ):
            # ---- constants ----
            w1t = cpool.tile([128, KT1 * HID], f32)
            w2t = cpool.tile([128, MT1 * L9], f32)
            b1c = cpool.tile([128, MT1], f32)
            b2c = cpool.tile([L9, 1], f32)
            trflat = cpool.tile([128, 81], f32)
            iota81 = cpool.tile([128, 81], f32)
            trcol9 = cpool.tile([BL, L9], f32)
            trow10 = cpool.tile([BL, L9], f32)
            iota9 = cpool.tile([BL, L9], f32)
            ident = cpool.tile([L9, L9], f32)
            for t_, d_ in [(w1t, w1t_d), (w2t, w2t_d), (b1c, b1_d),
                           (b2c, b2_d), (trflat, trflat_d), (iota81, iota81_d),
                           (trcol9, trcol9_d), (trow10, trow10_d),
                           (iota9, iota9_d), (ident, ident_d)]:
                nc.sync.dma_start(t_[:], d_[:])

            logitsD = dpool.tile([BL * T, L9], f32)

            # ---- MLP + transpose, chunked over 512 tokens ----
            for c in range(NCH):
                xt = xpool.tile([128, KT1 * 512], f32, tag="xt")
                for k in range(KT1):
                    nc.sync.dma_start(
                        xt[:, k * 512:(k + 1) * 512],
                        xt_d[k * 128:(k + 1) * 128, c * 512:(c + 1) * 512])
                hch = hpool.tile([128, MT1 * 512], f32, tag="h")
                for m in range(MT1):
                    pm = psum_mm.tile([128, 512], f32, tag="pm")
                    for k in range(KT1):
                        nc.tensor.matmul(
                            pm[:],
                            w1t[:, k * HID + m * 128: k * HID + (m + 1) * 128],
                            xt[:, k * 512:(k + 1) * 512],
                            start=(k == 0), stop=(k == KT1 - 1))
                    nc.scalar.activation(hch[:, m * 512:(m + 1) * 512], pm[:],
                                         ACT.Tanh, bias=b1c[:, m:m + 1],
                                         scale=1.0)
                lgp = psum_lg.tile([L9, 512], f32, tag="lgp")
                for m in range(MT1):
                    nc.tensor.matmul(
                        lgp[:], w2t[:, m * L9:(m + 1) * L9],
                        hch[:, m * 512:(m + 1) * 512],
                        start=(m == 0), stop=(m == MT1 - 1))
                lgs = lgpool.tile([L9, 512], f32, tag="lgs")
                nc.vector.tensor_scalar_add(lgs[:], lgp[:], b2c[:, 0:1])
                tpp = psum_tp.tile([128, 4 * L9], f32, tag="tpp")
                for j in range(4):
                    nc.tensor.transpose(tpp[:, j * L9:(j + 1) * L9],
                                        lgs[:, j * 128:(j + 1) * 128],
                                        ident[:])
                tps = lgpool.tile([128, 4 * L9], f32, tag="tps")
                nc.scalar.activation(tps[:], tpp[:], ACT.Copy,
                                     bias=0.0, scale=1.0)
                nc.sync.dma_start(
                    logitsD[c * 512:(c + 1) * 512, :]
                    .rearrange("(j p) l -> p j l", p=128),
                    tps[:].rearrange("p (j l) -> p j l", j=4))

            # ---- batch-major logit history ----
            lhist = spool.tile([BL, T * L9], f32)
            nc.sync.dma_start(lhist[:],
                              logitsD[:].rearrange("(b t) l -> b (t l)", b=BL))

            # ---- sequential Viterbi scan (9 states) ----
            whist = spool.tile([BL, (T + 1) * L9], f32)
            rhist = spool.tile([BL, T * L9], f32)
            vt0 = spool.tile([BL, 81], f32)
            nc.vector.memset(whist[:, 0:L9], 0.0)
            nc.vector.memset(rhist[:, 0:L9], 0.0)
            # W_1 = Tr[n, start] + logit_0
            nc.vector.tensor_tensor(whist[:, L9:2 * L9], lhist[:, 0:L9],
                                    trcol9[:], op=ADD)
            tr9v = trflat[0:BL, :].rearrange("b (n p) -> b n p", n=L9)
            vt0v = vt0[:].rearrange("b (n p) -> b n p", n=L9)
            for t in range(1, T):
                wsl = whist[:, t * L9:(t + 1) * L9]
                nc.vector.tensor_tensor(vt0v, _insert_bcast(wsl, 1, L9),
                                        tr9v, op=ADD)
                nc.vector.tensor_reduce(rhist[:, t * L9:(t + 1) * L9], vt0v,
                                        axis=AX, op=MAX)
                nc.vector.tensor_tensor(whist[:, (t + 1) * L9:(t + 2) * L9],
                                        rhist[:, t * L9:(t + 1) * L9],
                                        lhist[:, t * L9:(t + 1) * L9], op=ADD)

            # ---- final step: end transition + argmax ----
            wfin = spool.tile([BL, L9], f32)
            rf = spool.tile([BL, 1], f32)
            eqf = spool.tile([BL, L9], f32)
            ys = spool.tile([BL, T + 1], f32)
            nc.vector.tensor_tensor(wfin[:], whist[:, T * L9:(T + 1) * L9],
                                    trow10[:], op=ADD)
            nc.vector.tensor_reduce(rf[:], wfin[:], axis=AX, op=MAX)
            nc.vector.tensor_tensor(eqf[:], wfin[:],
                                    rf[:].broadcast_to((BL, L9)), op=EQ)
            nc.vector.tensor_tensor(eqf[:], eqf[:], iota9[:], op=MULT)
            nc.vector.tensor_reduce(ys[:, T:T + 1], eqf[:], axis=AX, op=MIN)

            # ---- batched backpointer pass on 128 partitions ----
            NS = 128 // BL                    # 16 t-slices
            TSL = T // NS                     # 32 steps per slice
            wp = spool.tile([128, TSL * L9], f32)
            rp = spool.tile([128, TSL * L9], f32)
            vtp = spool.tile([128, TSL * 81], f32)
            eqp = spool.tile([128, TSL * 81], f32)
            ptp = spool.tile([128, TSL * L9], f32)
            ptrh = spool.tile([BL, T * L9], f32)
            for s in range(NS):
                nc.sync.dma_start(wp[s * BL:(s + 1) * BL, :],
                                  whist[:, s * TSL * L9:(s + 1) * TSL * L9])
                nc.sync.dma_start(rp[s * BL:(s + 1) * BL, :],
                                  rhist[:, s * TSL * L9:(s + 1) * TSL * L9])
            wpv = _insert_bcast(
                wp[:].rearrange("q (t p) -> q t p", p=L9), 2, L9)
            trv = _insert_bcast(
                trflat[:].rearrange("q (n p) -> q n p", n=L9), 1, TSL)
            iov = _insert_bcast(
                iota81[:].rearrange("q (n p) -> q n p", n=L9), 1, TSL)
            vtpv = vtp[:].rearrange("q (t n p) -> q t n p", n=L9, p=L9)
            eqpv = eqp[:].rearrange("q (t n p) -> q t n p", n=L9, p=L9)
            rpv = rp[:].rearrange("q (t n) -> q t n", n=L9) \
                       .broadcast_to((128, TSL, L9, L9))
            nc.vector.tensor_tensor(vtpv, wpv, trv, op=ADD)
            nc.vector.tensor_tensor(eqpv, vtpv, rpv, op=EQ)
            nc.vector.tensor_tensor(vtpv, eqpv, iov, op=MULT)
            nc.vector.tensor_reduce(
                ptp[:].rearrange("q (t n) -> q t n", n=L9),
                vtpv, axis=AX, op=MIN)
            for s in range(NS):
                nc.sync.dma_start(ptrh[:, s * TSL * L9:(s + 1) * TSL * L9],
                                  ptp[s * BL:(s + 1) * BL, :])

            # ---- backtrack ----
            oh = spool.tile([BL, L9], f32)
            for t in range(T - 1, 0, -1):
                nc.vector.scalar_tensor_tensor(
                    oh[:], iota9[:], ys[:, t + 1:t + 2],
                    ptrh[:, t * L9:(t + 1) * L9],
                    op0=EQ, op1=MULT, accum_out=ys[:, t:t + 1])

            # ---- assemble preds ----
            predsF = spool.tile([BL, T], f32)
            predsI = spool.tile([BL, T], i32)
            nc.vector.tensor_scalar_add(predsF[:], ys[:, 1:T + 1], SHIFT)
            nc.vector.tensor_copy(predsI[:], predsF[:])
            nc.sync.dma_start(preds_d[:], predsI[:])

    nc.compile()
    return nc


def _host_inputs(inputs, W1, b1, W2, b2, transition):
    """Build the per-core in_maps."""
    W1T = np.ascontiguousarray(W1.T)                      # [768, 384]
    W2T = np.ascontiguousarray(W2.T)                      # [384, 9]
    w1t = np.ascontiguousarray(
        W1T.reshape(KT1, 128, HID).transpose(1, 0, 2).reshape(128, KT1 * HID))
    w2t = np.ascontiguousarray(
        W2T.reshape(MT1, 128, L9).transpose(1, 0, 2).reshape(128, MT1 * L9))
    b1c = np.ascontiguousarray(b1.reshape(MT1, 128).T)
    b2c = np.ascontiguousarray(b2.reshape(L9, 1))
    Tr9 = transition[:L9, :L9].astype(np.float32)
    trflat = np.tile(Tr9.reshape(1, 81), (128, 1))
    iota_p = np.arange(L9, dtype=np.float32) - SHIFT
    iota81 = np.tile(np.tile(iota_p, L9).reshape(1, 81), (128, 1))
    trcol9 = np.tile(transition[:L9, L9 + 0].reshape(1, L9), (BL, 1))
    trow10 = np.tile(transition[L9 + 1, :L9].reshape(1, L9), (BL, 1))
    iota9 = np.tile(iota_p.reshape(1, L9), (BL, 1))
    ident = np.eye(L9, dtype=np.float32)

    in_maps = []
    for c in range(NCORES):
        Xc = inputs[c * BL:(c + 1) * BL].reshape(BL * T, D)
        xt = np.ascontiguousarray(Xc.T)                  # [768, BL*T]
        in_maps.append({
            "xt": xt, "w1t": w1t, "w2t": w2t, "b1c": b1c, "b2c": b2c,
            "trflat": trflat, "iota81": iota81, "trcol9": trcol9,
            "trow10": trow10, "iota9": iota9, "ident": ident,
        })
    return in_maps


def kernel(inputs, labels_mask, W1, b1, W2, b2, transition):
    inputs = np.asarray(inputs, np.float32)
    labels_mask = np.asarray(labels_mask, np.int32)
    W1 = np.asarray(W1, np.float32)
    b1 = np.asarray(b1, np.float32)
    W2 = np.asarray(W2, np.float32)
    b2 = np.asarray(b2, np.float32)
    transition = np.asarray(transition, np.float32)

    if not np.all(labels_mask == 1):
        return _np_fallback(inputs, labels_mask, W1, b1, W2, b2, transition)

    from concourse import bass_utils

    if "nc" not in _cache:
        _cache["nc"] = _build_program()
    nc = _cache["nc"]

    in_maps = _host_inputs(inputs, W1, b1, W2, b2, transition)
    res = bass_utils.run_bass_kernel_spmd(
        nc, in_maps, core_ids=list(range(NCORES)))
    _cache["last_res"] = res
    preds = np.concatenate(
        [np.asarray(res.results[c]["preds"]) for c in range(NCORES)], axis=0)
    return preds.astype(np.int32)


if __name__ == "__main__":
    import reference
    ins = reference.setup_inputs()
    ins = {k: np.asarray(v) for k, v in ins.items()}
    out = kernel(**ins)
    print("kernel out", out.shape, out.dtype)


# revision 13
# speedup vs baseline: 1.2230x; 1.2230x over previous
"""CRF Viterbi decoder (MLP 768->384->9 + 11-state CRF) on 8 Trainium2 cores.

Strategy (data-parallel over batch, zero cross-core communication):
- Each core gets 8 of the 64 batches. MLP runs as PE matmuls (fp32) with
  tokens in the free dim; tanh/bias on the scalar engine.
- Logits are PE-transposed to token-major, bounced through DRAM, and
  reloaded batch-major [8, T*9] for the scan.
- The Viterbi scan is reduced to the 9 real states (start/end states can
  never win for t>=1; vit0 makes step 0 closed-form), run sequentially on
  the vector engine with exactly the reference's fp32 op ordering:
      vt0 = W_t[b,p] + Tr[n,p];  R_t = max_p vt0;  W_{t+1} = R_t + logit_t
- Backpointers are NOT computed in the scan; they are recomputed afterwards
  in one batched pass over all t on 128 partitions via
  eq = (vt0 == R) ; ptr~ = min_p(eq ? (p-16) : 0)   (first-max semantics).
- Backtrack: one fused scalar_tensor_tensor per step:
      onehot = (iota-16 == idx~) * ptr~_t ; idx~' = sum(onehot)
- preds = shifted indices + 16, cast to int32.
"""

import sys

if "/opt/trn_rl_repo" not in sys.path:
    sys.path.insert(0, "/opt/trn_rl_repo")

import numpy as np

B, T, D, HID, L9 = 64, 512, 768, 384, 9
NCORES = 8
BL = B // NCORES          # 8 batches per core
KT1 = D // 128            # 6 k-tiles, layer 1
MT1 = HID // 128          # 3 m-tiles, layer 1
NCH = (BL * T) // 512     # 8 token chunks of 512
SHIFT = 16.0              # index shift so sel values are < 0

_cache = {}


def _np_fallback(inputs, labels_mask, W1, b1, W2, b2, transition):
    """Pure-numpy reference (only used if labels_mask is not all ones)."""
    Bq, Tq, Dq = inputs.shape
    h = np.tanh(inputs.reshape(-1, Dq) @ W1.T + b1)
    logits = (h @ W2.T + b2).reshape(Bq, Tq, -1)
    NEG = -10000.0
    L = logits.shape[-1] + 2
    logits = np.concatenate(
        [logits, np.full((Bq, Tq, 2), NEG, np.float32)], -1)
    start, end = L - 2, L - 1
    lens = labels_mask.sum(-1)
    vit = np.full((Bq, L), NEG, np.float32)
    vit[:, start] = 0.0
    c = lens.astype(np.int64).copy()
    ptrs = np.zeros((Tq, Bq, L), np.int32)
    for t in range(Tq):
        vt = vit[:, None, :] + transition[None, :, :]
        ptrs[t] = vt.argmax(-1)
        vit_n = vt.max(-1) + logits[:, t]
        active = (c > 0)[:, None]
        vit_new = np.where(active, vit_n, vit)
        last = (c == 1)[:, None].astype(np.float32)
        vit = vit_new + last * transition[end][None, :]
        c -= 1
    idx = vit.argmax(-1).astype(np.int32)
    ys = np.zeros((Tq, Bq), np.int32)
    cur = idx
    for t in range(Tq - 1, -1, -1):
        cur = ptrs[t, np.arange(Bq), cur]
        ys[t] = cur
    preds = np.concatenate([ys[1:], idx[None]], 0).T
    return preds.astype(np.int32)


def _insert_bcast(ap, pos, count):
    from concourse import bass
    dims = list(ap.ap)
    dims.insert(pos, [0, count])
    return bass.AP(ap.tensor, ap.offset, dims)


def _build_program():
    from concourse import bass, bacc, tile, mybir

    f32 = mybir.dt.float32
    i32 = mybir.dt.int32
    ADD = mybir.AluOpType.add
    MULT = mybir.AluOpType.mult
    MAX = mybir.AluOpType.max
    MIN = mybir.AluOpType.min
    EQ = mybir.AluOpType.is_equal
    AX = mybir.AxisListType.X
    ACT = mybir.ActivationFunctionType

    nc = bacc.Bacc("TRN2", target_bir_lowering=False, debug=False,
                   num_devices=NCORES)

    xt_d = nc.dram_tensor("xt", [D, BL * T], f32, kind="ExternalInput")
    w1t_d = nc.dram_tensor("w1t", [128, KT1 * HID], f32, kind="ExternalInput")
    w2t_d = nc.dram_tensor("w2t", [128, MT1 * L9], f32, kind="ExternalInput")
    b1_d = nc.dram_tensor("b1c", [128, MT1], f32, kind="ExternalInput")
    b2_d = nc.dram_tensor("b2c", [L9, 1], f32, kind="ExternalInput")
    trflat_d = nc.dram_tensor("trflat", [128, 81], f32, kind="ExternalInput")
    iota81_d = nc.dram_tensor("iota81", [128, 81], f32, kind="ExternalInput")
    trcol9_d = nc.dram_tensor("trcol9", [BL, L9], f32, kind="ExternalInput")
    trow10_d = nc.dram_tensor("trow10", [BL, L9], f32, kind="ExternalInput")
    iota9_d = nc.dram_tensor("iota9", [BL, L9], f32, kind="ExternalInput")
    ident_d = nc.dram_tensor("ident", [L9, L9], f32, kind="ExternalInput")
    preds_d = nc.dram_tensor("preds", [BL, T], i32, kind="ExternalOutput")

    with tile.TileContext(nc) as tc:
        with (
            tc.tile_pool(name="const", bufs=1) as cpool,
            tc.tile_pool(name="xt", bufs=2) as xpool,
            tc.tile_pool(name="h", bufs=2) as hpool,
            tc.tile_pool(name="lg", bufs=2) as lgpool,
            tc.tile_pool(name="scan", bufs=1) as spool,
            tc.tile_pool(name="mm", bufs=4, space="PSUM") as psum_mm,
            tc.tile_pool(name="lgp", bufs=2, space="PSUM") as psum_lg,
            tc.tile_pool(name="tp", bufs=2, space="PSUM") as psum_tp,
        ):
            # ---- constants ----
            w1t = cpool.tile([128, KT1 * HID], f32)
            w2t = cpool.tile([128, MT1 * L9], f32)
            b1c = cpool.tile([128, MT1], f32)
            b2c = cpool.tile([L9, 1], f32)
            trflat = cpool.tile([128, 81], f32)
            iota81 = cpool.tile([128, 81], f32)
            trcol9 = cpool.tile([BL, L9], f32)
            trow10 = cpool.tile([BL, L9], f32)
            iota9 = cpool.tile([BL, L9], f32)
            ident = cpool.tile([L9, L9], f32)
            for t_, d_ in [(w1t, w1t_d), (w2t, w2t_d), (b1c, b1_d),
                           (b2c, b2_d), (trflat, trflat_d), (iota81, iota81_d),
                           (trcol9, trcol9_d), (trow10, trow10_d),
                           (iota9, iota9_d), (ident, ident_d)]:
                nc.sync.dma_start(t_[:], d_[:])

            # Per-chunk logit tiles, batch-major [8, 64*9]. Token order in
            # xt is (t, b): tok = t*BL + b, so chunk c covers t in
            # [c*TCH, (c+1)*TCH) for ALL batches -> the scan can start as
            # soon as chunk 0 lands.
            TCH = T // NCH                      # 64 steps per chunk
            lh = [spool.tile([BL, TCH * L9], f32, name=f"lh{c}",
                             tag=f"lh{c}")
                  for c in range(NCH)]

            # ---- MLP + transpose, chunked over 512 tokens ----
            for c in range(NCH):
                xt = xpool.tile([128, KT1 * 512], f32, tag="xt")
                for k in range(KT1):
                    nc.sync.dma_start(
                        xt[:, k * 512:(k + 1) * 512],
                        xt_d[k * 128:(k + 1) * 128, c * 512:(c + 1) * 512])
                hch = hpool.tile([128, MT1 * 512], f32, tag="h")
                for m in range(MT1):
                    pm = psum_mm.tile([128, 512], f32, tag="pm")
                    for k in range(KT1):
                        nc.tensor.matmul(
                            pm[:],
                            w1t[:, k * HID + m * 128: k * HID + (m + 1) * 128],
                            xt[:, k * 512:(k + 1) * 512],
                            start=(k == 0), stop=(k == KT1 - 1))
                    nc.scalar.activation(hch[:, m * 512:(m + 1) * 512], pm[:],
                                         ACT.Tanh, bias=b1c[:, m:m + 1],
                                         scale=1.0)
                lgp = psum_lg.tile([L9, 512], f32, tag="lgp")
                for m in range(MT1):
                    nc.tensor.matmul(
                        lgp[:], w2t[:, m * L9:(m + 1) * L9],
                        hch[:, m * 512:(m + 1) * 512],
                        start=(m == 0), stop=(m == MT1 - 1))
                lgs = lgpool.tile([L9, 512], f32, tag="lgs")
                nc.vector.tensor_scalar_add(lgs[:], lgp[:], b2c[:, 0:1])
                tpp = psum_tp.tile([128, 4 * L9], f32, tag="tpp")
                for j in range(4):
                    nc.tensor.transpose(tpp[:, j * L9:(j + 1) * L9],
                                        lgs[:, j * 128:(j + 1) * 128],
                                        ident[:])
                tps = lgpool.tile([128, 4 * L9], f32, tag="tps")
                nc.scalar.activation(tps[:], tpp[:], ACT.Copy,
                                     bias=0.0, scale=1.0)
                # Relayout [tok%128, (j, l)] -> [b, (t_local*9 + l)].
                # tok = c*512 + j*128 + p; t_local = j*16 + p//8; b = p%8.
                # One DMA per ph = p//8: partitions ph*8..ph*8+8 -> rows 0..8,
                # free dims (j: dst step 16*9, l).
                for ph in range(16):
                    sap = tps[ph * BL:(ph + 1) * BL, :]
                    src = bass.AP(sap.tensor, sap.offset,
                                  [list(sap.ap[0]), [L9, 4], [1, L9]])
                    dap = lh[c][:]
                    dst = bass.AP(dap.tensor, dap.offset + ph * L9,
                                  [list(dap.ap[0]), [16 * L9, 4], [1, L9]])
                    nc.sync.dma_start(dst, src)

            # ---- sequential Viterbi scan (9 states) ----
            whist = spool.tile([BL, (T + 1) * L9], f32)
            rhist = spool.tile([BL, T * L9], f32)
            vt0 = spool.tile([BL, 81], f32)
            nc.vector.memset(whist[:, 0:L9], 0.0)
            nc.vector.memset(rhist[:, 0:L9], 0.0)
            # W_1 = Tr[n, start] + logit_0
            nc.vector.tensor_tensor(whist[:, L9:2 * L9], lh[0][:, 0:L9],
                                    trcol9[:], op=ADD)
            tr9v = trflat[0:BL, :].rearrange("b (n p) -> b n p", n=L9)
            vt0v = vt0[:].rearrange("b (n p) -> b n p", n=L9)
            for t in range(1, T):
                wsl = whist[:, t * L9:(t + 1) * L9]
                nc.vector.tensor_tensor(vt0v, _insert_bcast(wsl, 1, L9),
                                        tr9v, op=ADD)
                nc.vector.tensor_reduce(rhist[:, t * L9:(t + 1) * L9], vt0v,
                                        axis=AX, op=MAX)
                tl = (t % TCH) * L9
                nc.vector.tensor_tensor(whist[:, (t + 1) * L9:(t + 2) * L9],
                                        rhist[:, t * L9:(t + 1) * L9],
                                        lh[t // TCH][:, tl:tl + L9], op=ADD)

            # ---- final step: end transition + argmax ----
            wfin = spool.tile([BL, L9], f32)
            rf = spool.tile([BL, 1], f32)
            eqf = spool.tile([BL, L9], f32)
            ys = spool.tile([BL, T + 1], f32)
            nc.vector.tensor_tensor(wfin[:], whist[:, T * L9:(T + 1) * L9],
                                    trow10[:], op=ADD)
            nc.vector.tensor_reduce(rf[:], wfin[:], axis=AX, op=MAX)
            nc.vector.tensor_tensor(eqf[:], wfin[:],
                                    rf[:].broadcast_to((BL, L9)), op=EQ)
            nc.vector.tensor_tensor(eqf[:], eqf[:], iota9[:], op=MULT)
            nc.vector.tensor_reduce(ys[:, T:T + 1], eqf[:], axis=AX, op=MIN)

            # ---- batched backpointer pass on 128 partitions ----
            NS = 128 // BL                    # 16 t-slices
            TSL = T // NS                     # 32 steps per slice
            wp = spool.tile([128, TSL * L9], f32)
            rp = spool.tile([128, TSL * L9], f32)
            vtp = spool.tile([128, TSL * 81], f32)
            eqp = spool.tile([128, TSL * 81], f32)
            ptp = spool.tile([128, TSL * L9], f32)
            ptrh = spool.tile([BL, T * L9], f32)
            for s in range(NS):
                nc.sync.dma_start(wp[s * BL:(s + 1) * BL, :],
                                  whist[:, s * TSL * L9:(s + 1) * TSL * L9])
                nc.sync.dma_start(rp[s * BL:(s + 1) * BL, :],
                                  rhist[:, s * TSL * L9:(s + 1) * TSL * L9])
            wpv = _insert_bcast(
                wp[:].rearrange("q (t p) -> q t p", p=L9), 2, L9)
            trv = _insert_bcast(
                trflat[:].rearrange("q (n p) -> q n p", n=L9), 1, TSL)
            iov = _insert_bcast(
                iota81[:].rearrange("q (n p) -> q n p", n=L9), 1, TSL)
            vtpv = vtp[:].rearrange("q (t n p) -> q t n p", n=L9, p=L9)
            eqpv = eqp[:].rearrange("q (t n p) -> q t n p", n=L9, p=L9)
            rpv = rp[:].rearrange("q (t n) -> q t n", n=L9) \
                       .broadcast_to((128, TSL, L9, L9))
            nc.vector.tensor_tensor(vtpv, wpv, trv, op=ADD)
            nc.vector.tensor_tensor(eqpv, vtpv, rpv, op=EQ)
            nc.vector.tensor_tensor(vtpv, eqpv, iov, op=MULT)
            nc.vector.tensor_reduce(
                ptp[:].rearrange("q (t n) -> q t n", n=L9),
                vtpv, axis=AX, op=MIN)
            for s in range(NS):
                nc.sync.dma_start(ptrh[:, s * TSL * L9:(s + 1) * TSL * L9],
                                  ptp[s * BL:(s + 1) * BL, :])

            # ---- backtrack ----
            oh = spool.tile([BL, L9], f32)
            for t in range(T - 1, 0, -1):
                nc.vector.scalar_tensor_tensor(
                    oh[:], iota9[:], ys[:, t + 1:t + 2],
                    ptrh[:, t * L9:(t + 1) * L9],
                    op0=EQ, op1=MULT, accum_out=ys[:, t:t + 1])

            # ---- assemble preds ----
            predsF = spool.tile([BL, T], f32)
            predsI = spool.tile([BL, T], i32)
            nc.vector.tensor_scalar_add(predsF[:], ys[:, 1:T + 1], SHIFT)
            nc.vector.tensor_copy(predsI[:], predsF[:])
            nc.sync.dma_start(preds_d[:], predsI[:])

    nc.compile()
    return nc


def _host_inputs(inputs, W1, b1, W2, b2, transition):
    """Build the per-core in_maps."""
    W1T = np.ascontiguousarray(W1.T)                      # [768, 384]
    W2T = np.ascontiguousarray(W2.T)                      # [384, 9]
    w1t = np.ascontiguousarray(
        W1T.reshape(KT1, 128, HID).transpose(1, 0, 2).reshape(128, KT1 * HID))
    w2t = np.ascontiguousarray(
        W2T.reshape(MT1, 128, L9).transpose(1, 0, 2).reshape(128, MT1 * L9))
    b1c = np.ascontiguousarray(b1.reshape(MT1, 128).T)
    b2c = np.ascontiguousarray(b2.reshape(L9, 1))
    Tr9 = transition[:L9, :L9].astype(np.float32)
    trflat = np.tile(Tr9.reshape(1, 81), (128, 1))
    iota_p = np.arange(L9, dtype=np.float32) - SHIFT
    iota81 = np.tile(np.tile(iota_p, L9).reshape(1, 81), (128, 1))
    trcol9 = np.tile(transition[:L9, L9 + 0].reshape(1, L9), (BL, 1))
    trow10 = np.tile(transition[L9 + 1, :L9].reshape(1, L9), (BL, 1))
    iota9 = np.tile(iota_p.reshape(1, L9), (BL, 1))
    ident = np.eye(L9, dtype=np.float32)

    in_maps = []
    for c in range(NCORES):
        # token order (t, b): tok = t*BL + b  -> chunk = contiguous t range
        Xc = inputs[c * BL:(c + 1) * BL].transpose(1, 0, 2).reshape(BL * T, D)
        xt = np.ascontiguousarray(Xc.T)                  # [768, BL*T]
        in_maps.append({
            "xt": xt, "w1t": w1t, "w2t": w2t, "b1c": b1c, "b2c": b2c,
            "trflat": trflat, "iota81": iota81, "trcol9": trcol9,
            "trow10": trow10, "iota9": iota9, "ident": ident,
        })
    return in_maps


def kernel(inputs, labels_mask, W1, b1, W2, b2, transition):
    inputs = np.asarray(inputs, np.float32)
    labels_mask = np.asarray(labels_mask, np.int32)
    W1 = np.asarray(W1, np.float32)
    b1 = np.asarray(b1, np.float32)
    W2 = np.asarray(W2, np.float32)
    b2 = np.asarray(b2, np.float32)
    transition = np.asarray(transition, np.float32)

    if not np.all(labels_mask == 1):
        return _np_fallback(inputs, labels_mask, W1, b1, W2, b2, transition)

    from concourse import bass_utils

    if "nc" not in _cache:
        _cache["nc"] = _build_program()
    nc = _cache["nc"]

    in_maps = _host_inputs(inputs, W1, b1, W2, b2, transition)
    res = bass_utils.run_bass_kernel_spmd(
        nc, in_maps, core_ids=list(range(NCORES)))
    _cache["last_res"] = res
    preds = np.concatenate(
        [np.asarray(res.results[c]["preds"]) for c in range(NCORES)], axis=0)
    return preds.astype(np.int32)


if __name__ == "__main__":
    import reference
    ins = reference.setup_inputs()
    ins = {k: np.asarray(v) for k, v in ins.items()}
    out = kernel(**ins)
    print("kernel out", out.shape, out.dtype)


# revision 23
# speedup vs baseline: 1.4155x; 1.1573x over previous
"""CRF Viterbi decoder (MLP 768->384->9 + 11-state CRF) on 8 Trainium2 cores.

Strategy (data-parallel over batch, zero cross-core communication):
- Each core gets 8 of the 64 batches. MLP runs as PE matmuls (fp32) with
  tokens in the free dim; tanh/bias on the scalar engine.
- Logits are PE-transposed to token-major, bounced through DRAM, and
  reloaded batch-major [8, T*9] for the scan.
- The Viterbi scan is reduced to the 9 real states (start/end states can
  never win for t>=1; vit0 makes step 0 closed-form), run sequentially on
  the vector engine with exactly the reference's fp32 op ordering:
      vt0 = W_t[b,p] + Tr[n,p];  R_t = max_p vt0;  W_{t+1} = R_t + logit_t
- Backpointers are NOT computed in the scan; they are recomputed afterwards
  in one batched pass over all t on 128 partitions via
  eq = (vt0 == R) ; ptr~ = min_p(eq ? (p-16) : 0)   (first-max semantics).
- Backtrack: one fused scalar_tensor_tensor per step:
      onehot = (iota-16 == idx~) * ptr~_t ; idx~' = sum(onehot)
- preds = shifted indices + 16, cast to int32.
"""

import sys

if "/opt/trn_rl_repo" not in sys.path:
    sys.path.insert(0, "/opt/trn_rl_repo")

import numpy as np

B, T, D, HID, L9 = 64, 512, 768, 384, 9
NCORES = 8
BL = B // NCORES          # 8 batches per core
KT1 = D // 128            # 6 k-tiles, layer 1
MT1 = HID // 128          # 3 m-tiles, layer 1
NCH = (BL * T) // 512     # 8 token chunks of 512
SHIFT = 16.0              # index shift so sel values are < 0

_cache = {}


def _np_fallback(inputs, labels_mask, W1, b1, W2, b2, transition):
    """Pure-numpy reference (only used if labels_mask is not all ones)."""
    Bq, Tq, Dq = inputs.shape
    h = np.tanh(inputs.reshape(-1, Dq) @ W1.T + b1)
    logits = (h @ W2.T + b2).reshape(Bq, Tq, -1)
    NEG = -10000.0
    L = logits.shape[-1] + 2
    logits = np.concatenate(
        [logits, np.full((Bq, Tq, 2), NEG, np.float32)], -1)
    start, end = L - 2, L - 1
    lens = labels_mask.sum(-1)
    vit = np.full((Bq, L), NEG, np.float32)
    vit[:, start] = 0.0
    c = lens.astype(np.int64).copy()
    ptrs = np.zeros((Tq, Bq, L), np.int32)
    for t in range(Tq):
        vt = vit[:, None, :] + transition[None, :, :]
        ptrs[t] = vt.argmax(-1)
        vit_n = vt.max(-1) + logits[:, t]
        active = (c > 0)[:, None]
        vit_new = np.where(active, vit_n, vit)
        last = (c == 1)[:, None].astype(np.float32)
        vit = vit_new + last * transition[end][None, :]
        c -= 1
    idx = vit.argmax(-1).astype(np.int32)
    ys = np.zeros((Tq, Bq), np.int32)
    cur = idx
    for t in range(Tq - 1, -1, -1):
        cur = ptrs[t, np.arange(Bq), cur]
        ys[t] = cur
    preds = np.concatenate([ys[1:], idx[None]], 0).T
    return preds.astype(np.int32)


def _insert_bcast(ap, pos, count):
    from concourse import bass
    dims = list(ap.ap)
    dims.insert(pos, [0, count])
    return bass.AP(ap.tensor, ap.offset, dims)


def _build_program():
    from concourse import bass, bacc, tile, mybir

    f32 = mybir.dt.float32
    i32 = mybir.dt.int32
    ADD = mybir.AluOpType.add
    MULT = mybir.AluOpType.mult
    MAX = mybir.AluOpType.max
    MIN = mybir.AluOpType.min
    EQ = mybir.AluOpType.is_equal
    AX = mybir.AxisListType.X
    ACT = mybir.ActivationFunctionType

    nc = bacc.Bacc("TRN2", target_bir_lowering=False, debug=False,
                   num_devices=NCORES)

    xt_d = nc.dram_tensor("xt", [D, BL * T], f32, kind="ExternalInput")
    w1t_d = nc.dram_tensor("w1t", [128, KT1 * HID], f32, kind="ExternalInput")
    w2t_d = nc.dram_tensor("w2t", [128, MT1 * L9], f32, kind="ExternalInput")
    b1_d = nc.dram_tensor("b1c", [128, MT1], f32, kind="ExternalInput")
    b2_d = nc.dram_tensor("b2c", [L9, 1], f32, kind="ExternalInput")
    trflat_d = nc.dram_tensor("trflat", [128, 81], f32, kind="ExternalInput")
    iota81_d = nc.dram_tensor("iota81", [128, 81], f32, kind="ExternalInput")
    trcol9_d = nc.dram_tensor("trcol9", [BL, L9], f32, kind="ExternalInput")
    trow10_d = nc.dram_tensor("trow10", [BL, L9], f32, kind="ExternalInput")
    iota9_d = nc.dram_tensor("iota9", [BL, L9], f32, kind="ExternalInput")
    ident_d = nc.dram_tensor("ident", [L9, L9], f32, kind="ExternalInput")
    b2row_d = nc.dram_tensor("b2row", [1, 4 * L9], f32, kind="ExternalInput")
    ones_d = nc.dram_tensor("ones1", [1, 128], f32, kind="ExternalInput")
    preds_d = nc.dram_tensor("preds", [BL, T], i32, kind="ExternalOutput")

    with tile.TileContext(nc) as tc:
        with (
            tc.tile_pool(name="const", bufs=1) as cpool,
            tc.tile_pool(name="xt", bufs=2) as xpool,
            tc.tile_pool(name="h", bufs=2) as hpool,
            tc.tile_pool(name="lg", bufs=2) as lgpool,
            tc.tile_pool(name="scan", bufs=1) as spool,
            tc.tile_pool(name="mm", bufs=4, space="PSUM") as psum_mm,
            tc.tile_pool(name="lgp", bufs=2, space="PSUM") as psum_lg,
            tc.tile_pool(name="tp", bufs=2, space="PSUM") as psum_tp,
        ):
            # ---- constants ----
            w1t = cpool.tile([128, KT1 * HID], f32)
            w2t = cpool.tile([128, MT1 * L9], f32)
            b1c = cpool.tile([128, MT1], f32)
            b2c = cpool.tile([L9, 1], f32)
            trflat = cpool.tile([128, 81], f32)
            iota81 = cpool.tile([128, 81], f32)
            trcol9 = cpool.tile([BL, L9], f32)
            trow10 = cpool.tile([BL, L9], f32)
            iota9 = cpool.tile([BL, L9], f32)
            ident = cpool.tile([L9, L9], f32)
            b2row = cpool.tile([1, 4 * L9], f32)
            ones1 = cpool.tile([1, 128], f32)
            for t_, d_ in [(w1t, w1t_d), (w2t, w2t_d), (b1c, b1_d),
                           (b2c, b2_d), (trflat, trflat_d), (iota81, iota81_d),
                           (trcol9, trcol9_d), (trow10, trow10_d),
                           (iota9, iota9_d), (ident, ident_d),
                           (b2row, b2row_d), (ones1, ones_d)]:
                nc.sync.dma_start(t_[:], d_[:])

            # Per-chunk logit tiles, batch-major [8, 64*9]. Token order in
            # xt is (t, b): tok = t*BL + b, so chunk c covers t in
            # [c*TCH, (c+1)*TCH) for ALL batches -> the scan can start as
            # soon as chunk 0 lands.
            TCH = T // NCH                      # 64 steps per chunk
            lh = [spool.tile([BL, TCH * L9], f32, name=f"lh{c}",
                             tag=f"lh{c}")
                  for c in range(NCH)]

            # ---- MLP + transpose, chunked over 512 tokens ----
            for c in range(NCH):
                xt = xpool.tile([128, KT1 * 512], f32, tag="xt")
                for k in range(KT1):
                    nc.sync.dma_start(
                        xt[:, k * 512:(k + 1) * 512],
                        xt_d[k * 128:(k + 1) * 128, c * 512:(c + 1) * 512])
                hch = hpool.tile([128, MT1 * 512], f32, tag="h")
                for m in range(MT1):
                    pm = psum_mm.tile([128, 512], f32, tag="pm")
                    for k in range(KT1):
                        nc.tensor.matmul(
                            pm[:],
                            w1t[:, k * HID + m * 128: k * HID + (m + 1) * 128],
                            xt[:, k * 512:(k + 1) * 512],
                            start=(k == 0), stop=(k == KT1 - 1))
                    nc.scalar.activation(hch[:, m * 512:(m + 1) * 512], pm[:],
                                         ACT.Tanh, bias=b1c[:, m:m + 1],
                                         scale=1.0)
                lgp = psum_lg.tile([L9, 512], f32, tag="lgp")
                for m in range(MT1):
                    nc.tensor.matmul(
                        lgp[:], w2t[:, m * L9:(m + 1) * L9],
                        hch[:, m * 512:(m + 1) * 512],
                        start=(m == 0), stop=(m == MT1 - 1))
                lgs = lgpool.tile([L9, 512], f32, tag="lgs")
                nc.scalar.activation(lgs[:], lgp[:], ACT.Identity,
                                     bias=b2c[:, 0:1], scale=1.0)
                tpp = psum_tp.tile([128, 4 * L9], f32, tag="tpp")
                for j in range(4):
                    nc.tensor.transpose(tpp[:, j * L9:(j + 1) * L9],
                                        lgs[:, j * 128:(j + 1) * 128],
                                        ident[:])
                tps = lgpool.tile([128, 4 * L9], f32, tag="tps")
                nc.scalar.activation(tps[:], tpp[:], ACT.Copy,
                                     bias=0.0, scale=1.0)
                # Relayout [tok%128, (j, l)] -> [b, (t_local*9 + l)].
                # tok = c*512 + j*128 + p; t_local = j*16 + p//8; b = p%8.
                # One DMA per ph = p//8: partitions ph*8..ph*8+8 -> rows 0..8,
                # free dims (j: dst step 16*9, l).
                for ph in range(16):
                    sap = tps[ph * BL:(ph + 1) * BL, :]
                    src = bass.AP(sap.tensor, sap.offset,
                                  [list(sap.ap[0]), [L9, 4], [1, L9]])
                    dap = lh[c][:]
                    dst = bass.AP(dap.tensor, dap.offset + ph * L9,
                                  [list(dap.ap[0]), [16 * L9, 4], [1, L9]])
                    nc.sync.dma_start(dst, src)

            # ---- sequential Viterbi scan (9 states) ----
            whist = spool.tile([BL, (T + 1) * L9], f32)
            rhist = spool.tile([BL, T * L9], f32)
            vt0 = spool.tile([BL, 81], f32)
            nc.vector.memset(whist[:, 0:L9], 0.0)
            nc.vector.memset(rhist[:, 0:L9], 0.0)
            # W_1 = Tr[n, start] + logit_0
            nc.vector.tensor_tensor(whist[:, L9:2 * L9], lh[0][:, 0:L9],
                                    trcol9[:], op=ADD)
            tr9v = trflat[0:BL, :].rearrange("b (n p) -> b n p", n=L9)
            vt0v = vt0[:].rearrange("b (n p) -> b n p", n=L9)
            for t in range(1, T):
                wsl = whist[:, t * L9:(t + 1) * L9]
                nc.vector.tensor_tensor(vt0v, _insert_bcast(wsl, 1, L9),
                                        tr9v, op=ADD)
                nc.vector.tensor_reduce(rhist[:, t * L9:(t + 1) * L9], vt0v,
                                        axis=AX, op=MAX)
                tl = (t % TCH) * L9
                nc.vector.tensor_tensor(whist[:, (t + 1) * L9:(t + 2) * L9],
                                        rhist[:, t * L9:(t + 1) * L9],
                                        lh[t // TCH][:, tl:tl + L9], op=ADD)

            # ---- final step: end transition + argmax ----
            wfin = spool.tile([BL, L9], f32)
            rf = spool.tile([BL, 1], f32)
            eqf = spool.tile([BL, L9], f32)
            ys = spool.tile([BL, T + 1], f32)
            nc.vector.tensor_tensor(wfin[:], whist[:, T * L9:(T + 1) * L9],
                                    trow10[:], op=ADD)
            nc.vector.tensor_reduce(rf[:], wfin[:], axis=AX, op=MAX)
            nc.vector.tensor_tensor(eqf[:], wfin[:],
                                    rf[:].broadcast_to((BL, L9)), op=EQ)
            nc.vector.tensor_tensor(eqf[:], eqf[:], iota9[:], op=MULT)
            nc.vector.tensor_reduce(ys[:, T:T + 1], eqf[:], axis=AX, op=MIN)

            # ---- batched backpointer pass on 128 partitions ----
            NS = 128 // BL                    # 16 t-slices
            TSL = T // NS                     # 32 steps per slice
            wp = spool.tile([128, TSL * L9], f32)
            rp = spool.tile([128, TSL * L9], f32)
            vtp = spool.tile([128, TSL * 81], f32)
            eqp = spool.tile([128, TSL * 81], f32)
            ptp = spool.tile([128, TSL * L9], f32)
            ptrh = spool.tile([BL, T * L9], f32)
            for s in range(NS):
                nc.sync.dma_start(wp[s * BL:(s + 1) * BL, :],
                                  whist[:, s * TSL * L9:(s + 1) * TSL * L9])
                nc.sync.dma_start(rp[s * BL:(s + 1) * BL, :],
                                  rhist[:, s * TSL * L9:(s + 1) * TSL * L9])
            wpv = _insert_bcast(
                wp[:].rearrange("q (t p) -> q t p", p=L9), 2, L9)
            trv = _insert_bcast(
                trflat[:].rearrange("q (n p) -> q n p", n=L9), 1, TSL)
            iov = _insert_bcast(
                iota81[:].rearrange("q (n p) -> q n p", n=L9), 1, TSL)
            vtpv = vtp[:].rearrange("q (t n p) -> q t n p", n=L9, p=L9)
            eqpv = eqp[:].rearrange("q (t n p) -> q t n p", n=L9, p=L9)
            rpv = rp[:].rearrange("q (t n) -> q t n", n=L9) \
                       .broadcast_to((128, TSL, L9, L9))
            nc.vector.tensor_tensor(vtpv, wpv, trv, op=ADD)
            nc.vector.tensor_tensor(eqpv, vtpv, rpv, op=EQ)
            nc.vector.tensor_tensor(vtpv, eqpv, iov, op=MULT)
            nc.vector.tensor_reduce(
                ptp[:].rearrange("q (t n) -> q t n", n=L9),
                vtpv, axis=AX, op=MIN)
            for s in range(NS):
                nc.sync.dma_start(ptrh[:, s * TSL * L9:(s + 1) * TSL * L9],
                                  ptp[s * BL:(s + 1) * BL, :])

            # ---- pointer-doubling backtrack ----
            # Compose disjoint pairs of pointer tables (exact integer maps):
            #   C1[tau] = J0[2tau] o J0[2tau+1]   (maps idx_{2tau+2} -> idx_{2tau})
            #   C2[u]   = C1[2u]  o C1[2u+1]      (maps idx_{4u+4} -> idx_{4u})
            #   C3[v]   = C2[2v]  o C2[2v+1]      (maps idx_{8v+8} -> idx_{8v})
            # then run a 63-step chain on C3 and recover skipped steps with
            # batched one-hot gathers.
            def compose(dst, srcT, npairs):
                """dst[q, pair, j] = srcT[q, 2*pair, iv(srcT[q, 2*pair+1, j])]
                in pass layout; srcT has 2*npairs tables of 9 per partition."""
                eqv = bass.AP(eqbufP[:].tensor, eqbufP[:].offset,
                              [list(eqbufP[:].ap[0]), [81, npairs],
                               [L9, L9], [1, L9]])
                odd = bass.AP(srcT.tensor, srcT.offset + L9,
                              [list(srcT.ap[0]), [2 * L9, npairs],
                               [1, L9], [0, L9]])
                iov_ = bass.AP(iota81[:].tensor, iota81[:].offset,
                               [list(iota81[:].ap[0]), [0, npairs],
                                [L9, L9], [1, L9]])
                nc.vector.tensor_tensor(eqv, odd, iov_, op=EQ)
                even = bass.AP(srcT.tensor, srcT.offset,
                               [list(srcT.ap[0]), [2 * L9, npairs],
                                [0, L9], [1, L9]])
                selv = bass.AP(selbufP[:].tensor, selbufP[:].offset,
                               [list(selbufP[:].ap[0]), [81, npairs],
                                [L9, L9], [1, L9]])
                nc.vector.tensor_tensor(selv, eqv, even, op=MULT)
                nc.vector.tensor_reduce(
                    bass.AP(dst.tensor, dst.offset,
                            [list(dst.ap[0]), [L9, npairs], [1, L9]]),
                    selv, axis=AX, op=ADD)

            eqbufP = spool.tile([128, 16 * 81], f32)
            selbufP = spool.tile([128, 16 * 81], f32)
            c1p = spool.tile([128, 16 * L9], f32)
            c2p = spool.tile([128, 8 * L9], f32)
            c3p = spool.tile([128, 4 * L9], f32)
            compose(c1p[:], ptp[:], 16)
            compose(c2p[:], c1p[:], 8)
            compose(c3p[:], c2p[:], 4)

            # relayout compositions to batch-major
            c1h = spool.tile([BL, 256 * L9], f32)
            c2h = spool.tile([BL, 128 * L9], f32)
            c3h = spool.tile([BL, 64 * L9], f32)
            for s in range(NS):
                for ch, cp, w in ((c1h, c1p, 16 * L9), (c2h, c2p, 8 * L9),
                                  (c3h, c3p, 4 * L9)):
                    nc.sync.dma_start(ch[:, s * w:(s + 1) * w],
                                      cp[s * BL:(s + 1) * BL, :])

            # 63-step chain on C3: idx_{8v} = C3[v][idx_{8v+8}]
            oh = spool.tile([BL, L9], f32)
            for v in range(64 - 1, 0, -1):
                nc.vector.scalar_tensor_tensor(
                    oh[:], iota9[:], ys[:, 8 * v + 8:8 * v + 9],
                    c3h[:, v * L9:(v + 1) * L9],
                    op0=EQ, op1=MULT, accum_out=ys[:, 8 * v:8 * v + 1])

            # batched recoveries, coarsest to finest
            eqb = spool.tile([BL, 256 * L9], f32)
            selb = spool.tile([BL, 256 * L9], f32)

            def recover(table, cnt, in_off, in_step, tab_off, tab_step,
                        out_off, out_step):
                """ys[out] = table_odd[ys[in]] for cnt positions at once."""
                eqv = bass.AP(eqb[:].tensor, eqb[:].offset,
                              [list(eqb[:].ap[0]), [L9, cnt], [1, L9]])
                io_b = bass.AP(iota9[:].tensor, iota9[:].offset,
                               [list(iota9[:].ap[0]), [0, cnt], [1, L9]])
                ys_in = bass.AP(ys[:].tensor, ys[:].offset + in_off,
                                [list(ys[:].ap[0]), [in_step, cnt], [0, L9]])
                nc.vector.tensor_tensor(eqv, io_b, ys_in, op=EQ)
                selv = bass.AP(selb[:].tensor, selb[:].offset,
                               [list(selb[:].ap[0]), [L9, cnt], [1, L9]])
                tab = bass.AP(table.tensor, table.offset + tab_off,
                              [list(table.ap[0]), [tab_step, cnt], [1, L9]])
                nc.vector.tensor_tensor(selv, eqv, tab, op=MULT)
                ys_out = bass.AP(ys[:].tensor, ys[:].offset + out_off,
                                 [list(ys[:].ap[0]), [out_step, cnt]])
                nc.vector.tensor_reduce(
                    ys_out,
                    bass.AP(selb[:].tensor, selb[:].offset,
                            [list(selb[:].ap[0]), [L9, cnt], [1, L9]]),
                    axis=AX, op=ADD)

            # level-2: idx_{8v+4} = C2[2v+1][idx_{8v+8}], v=0..63
            recover(c2h[:], 64, 8, 8, L9, 2 * L9, 4, 8)
            # level-1: idx_{4u+2} = C1[2u+1][idx_{4u+4}], u=0..127
            recover(c1h[:], 128, 4, 4, L9, 2 * L9, 2, 4)
            # level-0: idx_{2tau+1} = J0[2tau+1][idx_{2tau+2}], tau=0..255
            recover(ptrh[:], 256, 2, 2, L9, 2 * L9, 1, 2)

            # ---- assemble preds ----
            predsF = spool.tile([BL, T], f32)
            predsI = spool.tile([BL, T], i32)
            nc.vector.tensor_scalar_add(predsF[:], ys[:, 1:T + 1], SHIFT)
            nc.vector.tensor_copy(predsI[:], predsF[:])
            nc.sync.dma_start(preds_d[:], predsI[:])

    nc.compile()
    return nc


def _host_inputs(inputs, W1, b1, W2, b2, transition):
    """Build the per-core in_maps."""
    W1T = np.ascontiguousarray(W1.T)                      # [768, 384]
    W2T = np.ascontiguousarray(W2.T)                      # [384, 9]
    w1t = np.ascontiguousarray(
        W1T.reshape(KT1, 128, HID).transpose(1, 0, 2).reshape(128, KT1 * HID))
    w2t = np.ascontiguousarray(
        W2T.reshape(MT1, 128, L9).transpose(1, 0, 2).reshape(128, MT1 * L9))
    b1c = np.ascontiguousarray(b1.reshape(MT1, 128).T)
    b2c = np.ascontiguousarray(b2.reshape(L9, 1))
    Tr9 = transition[:L9, :L9].astype(np.float32)
    trflat = np.tile(Tr9.reshape(1, 81), (128, 1))
    iota_p = np.arange(L9, dtype=np.float32) - SHIFT
    iota81 = np.tile(np.tile(iota_p, L9).reshape(1, 81), (128, 1))
    trcol9 = np.tile(transition[:L9, L9 + 0].reshape(1, L9), (BL, 1))
    trow10 = np.tile(transition[L9 + 1, :L9].reshape(1, L9), (BL, 1))
    iota9 = np.tile(iota_p.reshape(1, L9), (BL, 1))
    ident = np.eye(L9, dtype=np.float32)
    b2row = np.tile(b2.reshape(1, L9), (1, 4)).astype(np.float32)
    ones1 = np.ones((1, 128), np.float32)

    in_maps = []
    for c in range(NCORES):
        # token order (t, b): tok = t*BL + b  -> chunk = contiguous t range
        Xc = inputs[c * BL:(c + 1) * BL].transpose(1, 0, 2).reshape(BL * T, D)
        xt = np.ascontiguousarray(Xc.T)                  # [768, BL*T]
        in_maps.append({
            "xt": xt, "w1t": w1t, "w2t": w2t, "b1c": b1c, "b2c": b2c,
            "trflat": trflat, "iota81": iota81, "trcol9": trcol9,
            "trow10": trow10, "iota9": iota9, "ident": ident,
            "b2row": b2row, "ones1": ones1,
        })
    return in_maps


def kernel(inputs, labels_mask, W1, b1, W2, b2, transition):
    inputs = np.asarray(inputs, np.float32)
    labels_mask = np.asarray(labels_mask, np.int32)
    W1 = np.asarray(W1, np.float32)
    b1 = np.asarray(b1, np.float32)
    W2 = np.asarray(W2, np.float32)
    b2 = np.asarray(b2, np.float32)
    transition = np.asarray(transition, np.float32)

    if not np.all(labels_mask == 1):
        return _np_fallback(inputs, labels_mask, W1, b1, W2, b2, transition)

    from concourse import bass_utils

    if "nc" not in _cache:
        _cache["nc"] = _build_program()
    nc = _cache["nc"]

    in_maps = _host_inputs(inputs, W1, b1, W2, b2, transition)
    res = bass_utils.run_bass_kernel_spmd(
        nc, in_maps, core_ids=list(range(NCORES)))
    _cache["last_res"] = res
    preds = np.concatenate(
        [np.asarray(res.results[c]["preds"]) for c in range(NCORES)], axis=0)
    return preds.astype(np.int32)


if __name__ == "__main__":
    import reference
    ins = reference.setup_inputs()
    ins = {k: np.asarray(v) for k, v in ins.items()}
    out = kernel(**ins)
    print("kernel out", out.shape, out.dtype)


# revision 30
# speedup vs baseline: 1.4682x; 1.0373x over previous
"""CRF Viterbi decoder (MLP 768->384->9 + 11-state CRF) on 8 Trainium2 cores.

Strategy (data-parallel over batch, zero cross-core communication):
- Each core gets 8 of the 64 batches. MLP runs as PE matmuls (fp32) with
  tokens in the free dim; tanh/bias on the scalar engine.
- Logits are PE-transposed to token-major, bounced through DRAM, and
  reloaded batch-major [8, T*9] for the scan.
- The Viterbi scan is reduced to the 9 real states (start/end states can
  never win for t>=1; vit0 makes step 0 closed-form), run sequentially on
  the vector engine with exactly the reference's fp32 op ordering:
      vt0 = W_t[b,p] + Tr[n,p];  R_t = max_p vt0;  W_{t+1} = R_t + logit_t
- Backpointers are NOT computed in the scan; they are recomputed afterwards
  in one batched pass over all t on 128 partitions via
  eq = (vt0 == R) ; ptr~ = min_p(eq ? (p-16) : 0)   (first-max semantics).
- Backtrack: one fused scalar_tensor_tensor per step:
      onehot = (iota-16 == idx~) * ptr~_t ; idx~' = sum(onehot)
- preds = shifted indices + 16, cast to int32.
"""

import sys

if "/opt/trn_rl_repo" not in sys.path:
    sys.path.insert(0, "/opt/trn_rl_repo")

import numpy as np

B, T, D, HID, L9 = 64, 512, 768, 384, 9
NCORES = 8
BL = B // NCORES          # 8 batches per core
KT1 = D // 128            # 6 k-tiles, layer 1
MT1 = HID // 128          # 3 m-tiles, layer 1
NCH = (BL * T) // 512     # 8 token chunks of 512
SHIFT = 16.0              # index shift so sel values are < 0

_cache = {}


def _np_fallback(inputs, labels_mask, W1, b1, W2, b2, transition):
    """Pure-numpy reference (only used if labels_mask is not all ones)."""
    Bq, Tq, Dq = inputs.shape
    h = np.tanh(inputs.reshape(-1, Dq) @ W1.T + b1)
    logits = (h @ W2.T + b2).reshape(Bq, Tq, -1)
    NEG = -10000.0
    L = logits.shape[-1] + 2
    logits = np.concatenate(
        [logits, np.full((Bq, Tq, 2), NEG, np.float32)], -1)
    start, end = L - 2, L - 1
    lens = labels_mask.sum(-1)
    vit = np.full((Bq, L), NEG, np.float32)
    vit[:, start] = 0.0
    c = lens.astype(np.int64).copy()
    ptrs = np.zeros((Tq, Bq, L), np.int32)
    for t in range(Tq):
        vt = vit[:, None, :] + transition[None, :, :]
        ptrs[t] = vt.argmax(-1)
        vit_n = vt.max(-1) + logits[:, t]
        active = (c > 0)[:, None]
        vit_new = np.where(active, vit_n, vit)
        last = (c == 1)[:, None].astype(np.float32)
        vit = vit_new + last * transition[end][None, :]
        c -= 1
    idx = vit.argmax(-1).astype(np.int32)
    ys = np.zeros((Tq, Bq), np.int32)
    cur = idx
    for t in range(Tq - 1, -1, -1):
        cur = ptrs[t, np.arange(Bq), cur]
        ys[t] = cur
    preds = np.concatenate([ys[1:], idx[None]], 0).T
    return preds.astype(np.int32)


def _insert_bcast(ap, pos, count):
    from concourse import bass
    dims = list(ap.ap)
    dims.insert(pos, [0, count])
    return bass.AP(ap.tensor, ap.offset, dims)


def _build_program():
    from concourse import bass, bacc, tile, mybir

    f32 = mybir.dt.float32
    i32 = mybir.dt.int32
    ADD = mybir.AluOpType.add
    MULT = mybir.AluOpType.mult
    MAX = mybir.AluOpType.max
    MIN = mybir.AluOpType.min
    EQ = mybir.AluOpType.is_equal
    AX = mybir.AxisListType.X
    ACT = mybir.ActivationFunctionType

    nc = bacc.Bacc("TRN2", target_bir_lowering=False, debug=False,
                   num_devices=NCORES)

    xt_d = nc.dram_tensor("xt", [D, BL * T], f32, kind="ExternalInput")
    w1t_d = nc.dram_tensor("w1t", [128, KT1 * HID], f32, kind="ExternalInput")
    w2t_d = nc.dram_tensor("w2t", [128, MT1 * L9], f32, kind="ExternalInput")
    b1_d = nc.dram_tensor("b1c", [128, MT1], f32, kind="ExternalInput")
    b2_d = nc.dram_tensor("b2c", [L9, 1], f32, kind="ExternalInput")
    trflat_d = nc.dram_tensor("trflat", [128, 81], f32, kind="ExternalInput")
    iota81_d = nc.dram_tensor("iota81", [128, 81], f32, kind="ExternalInput")
    trcol9_d = nc.dram_tensor("trcol9", [BL, L9], f32, kind="ExternalInput")
    trow10_d = nc.dram_tensor("trow10", [BL, L9], f32, kind="ExternalInput")
    iota9_d = nc.dram_tensor("iota9", [BL, L9], f32, kind="ExternalInput")
    ident_d = nc.dram_tensor("ident", [L9, L9], f32, kind="ExternalInput")
    b2row_d = nc.dram_tensor("b2row", [1, 4 * L9], f32, kind="ExternalInput")
    ones_d = nc.dram_tensor("ones1", [1, 128], f32, kind="ExternalInput")
    preds_d = nc.dram_tensor("preds", [BL, T], i32, kind="ExternalOutput")

    with tile.TileContext(nc) as tc:
        with (
            tc.tile_pool(name="const", bufs=1) as cpool,
            tc.tile_pool(name="xt", bufs=2) as xpool,
            tc.tile_pool(name="h", bufs=2) as hpool,
            tc.tile_pool(name="lg", bufs=2) as lgpool,
            tc.tile_pool(name="scan", bufs=1) as spool,
            tc.tile_pool(name="mm", bufs=4, space="PSUM") as psum_mm,
            tc.tile_pool(name="lgp", bufs=2, space="PSUM") as psum_lg,
            tc.tile_pool(name="tp", bufs=2, space="PSUM") as psum_tp,
        ):
            # ---- constants ----
            w1t = cpool.tile([128, KT1 * HID], f32)
            w2t = cpool.tile([128, MT1 * L9], f32)
            b1c = cpool.tile([128, MT1], f32)
            b2c = cpool.tile([L9, 1], f32)
            trflat = cpool.tile([128, 81], f32)
            iota81 = cpool.tile([128, 81], f32)
            trcol9 = cpool.tile([BL, L9], f32)
            trow10 = cpool.tile([BL, L9], f32)
            iota9 = cpool.tile([BL, L9], f32)
            ident = cpool.tile([L9, L9], f32)
            b2row = cpool.tile([1, 4 * L9], f32)
            ones1 = cpool.tile([1, 128], f32)
            for t_, d_ in [(w1t, w1t_d), (w2t, w2t_d), (b1c, b1_d),
                           (b2c, b2_d), (trflat, trflat_d), (iota81, iota81_d),
                           (trcol9, trcol9_d), (trow10, trow10_d),
                           (iota9, iota9_d), (ident, ident_d),
                           (b2row, b2row_d), (ones1, ones_d)]:
                nc.sync.dma_start(t_[:], d_[:])

            # Per-chunk logit tiles, batch-major. Token order in xt is
            # (t, b): tok = t*BL + b. Graduated chunk sizes so the scan can
            # start early while the MLP still streams.
            CH_STEPS = [16, 16, 32, 64, 64, 64, 64, 64, 64, 64]
            assert sum(CH_STEPS) == T
            CH_T0 = np.cumsum([0] + CH_STEPS).tolist()
            lh = [spool.tile([BL, tch * L9], f32, name=f"lh{c}",
                             tag=f"lh{c}")
                  for c, tch in enumerate(CH_STEPS)]
            t2chunk = []
            for c, tch in enumerate(CH_STEPS):
                t2chunk += [(c, t_ - CH_T0[c]) for t_ in
                            range(CH_T0[c], CH_T0[c] + tch)]

            # ---- MLP + transpose, chunked ----
            for c, tch in enumerate(CH_STEPS):
                ntok = tch * BL            # tokens in chunk (<= 512)
                nj = ntok // 128           # transpose tiles
                tok0 = CH_T0[c] * BL
                xt = xpool.tile([128, KT1 * 512], f32, tag="xt")
                for k in range(KT1):
                    nc.sync.dma_start(
                        xt[:, k * 512:k * 512 + ntok],
                        xt_d[k * 128:(k + 1) * 128, tok0:tok0 + ntok])
                hch = hpool.tile([128, MT1 * 512], f32, tag="h")
                for m in range(MT1):
                    pm = psum_mm.tile([128, 512], f32, tag="pm")
                    for k in range(KT1):
                        nc.tensor.matmul(
                            pm[:, 0:ntok],
                            w1t[:, k * HID + m * 128: k * HID + (m + 1) * 128],
                            xt[:, k * 512:k * 512 + ntok],
                            start=(k == 0), stop=(k == KT1 - 1))
                    nc.scalar.activation(hch[:, m * 512:m * 512 + ntok],
                                         pm[:, 0:ntok],
                                         ACT.Tanh, bias=b1c[:, m:m + 1],
                                         scale=1.0)
                lgp = psum_lg.tile([L9, 512], f32, tag="lgp")
                for m in range(MT1):
                    nc.tensor.matmul(
                        lgp[:, 0:ntok], w2t[:, m * L9:(m + 1) * L9],
                        hch[:, m * 512:m * 512 + ntok],
                        start=(m == 0), stop=(m == MT1 - 1))
                lgs = lgpool.tile([L9, 512], f32, tag="lgs")
                nc.scalar.activation(lgs[:, 0:ntok], lgp[:, 0:ntok],
                                     ACT.Identity,
                                     bias=b2c[:, 0:1], scale=1.0)
                tpp = psum_tp.tile([128, 4 * L9], f32, tag="tpp")
                for j in range(nj):
                    nc.tensor.transpose(tpp[:, j * L9:(j + 1) * L9],
                                        lgs[:, j * 128:(j + 1) * 128],
                                        ident[:])
                tps = lgpool.tile([128, 4 * L9], f32, tag="tps")
                nc.scalar.activation(tps[:, 0:nj * L9], tpp[:, 0:nj * L9],
                                     ACT.Copy, bias=0.0, scale=1.0)
                # Relayout [tok%128, (j, l)] -> [b, (t_local*9 + l)].
                # tok = tok0 + j*128 + p; t_local = j*16 + p//8; b = p%8.
                for ph in range(16):
                    sap = tps[ph * BL:(ph + 1) * BL, :]
                    src = bass.AP(sap.tensor, sap.offset,
                                  [list(sap.ap[0]), [L9, nj], [1, L9]])
                    dap = lh[c][:]
                    dst = bass.AP(dap.tensor, dap.offset + ph * L9,
                                  [list(dap.ap[0]), [16 * L9, nj], [1, L9]])
                    nc.sync.dma_start(dst, src)

            # ---- sequential Viterbi scan (9 states) ----
            whist = spool.tile([BL, (T + 1) * L9], f32)
            rhist = spool.tile([BL, T * L9], f32)
            vt0 = spool.tile([BL, 81], f32)
            nc.vector.memset(whist[:, 0:L9], 0.0)
            nc.vector.memset(rhist[:, 0:L9], 0.0)
            # W_1 = Tr[n, start] + logit_0
            nc.vector.tensor_tensor(whist[:, L9:2 * L9], lh[0][:, 0:L9],
                                    trcol9[:], op=ADD)
            tr9v = trflat[0:BL, :].rearrange("b (n p) -> b n p", n=L9)
            vt0v = vt0[:].rearrange("b (n p) -> b n p", n=L9)
            for t in range(1, T):
                wsl = whist[:, t * L9:(t + 1) * L9]
                nc.vector.tensor_tensor(vt0v, _insert_bcast(wsl, 1, L9),
                                        tr9v, op=ADD)
                nc.vector.tensor_reduce(rhist[:, t * L9:(t + 1) * L9], vt0v,
                                        axis=AX, op=MAX)
                tc_, tl_ = t2chunk[t]
                nc.vector.tensor_tensor(whist[:, (t + 1) * L9:(t + 2) * L9],
                                        rhist[:, t * L9:(t + 1) * L9],
                                        lh[tc_][:, tl_ * L9:(tl_ + 1) * L9],
                                        op=ADD)

            # ---- final step: end transition + argmax ----
            wfin = spool.tile([BL, L9], f32)
            rf = spool.tile([BL, 1], f32)
            eqf = spool.tile([BL, L9], f32)
            ys = spool.tile([BL, T + 1], f32)
            nc.vector.tensor_tensor(wfin[:], whist[:, T * L9:(T + 1) * L9],
                                    trow10[:], op=ADD)
            nc.vector.tensor_reduce(rf[:], wfin[:], axis=AX, op=MAX)
            nc.vector.tensor_tensor(eqf[:], wfin[:],
                                    rf[:].broadcast_to((BL, L9)), op=EQ)
            nc.vector.tensor_tensor(eqf[:], eqf[:], iota9[:], op=MULT)
            nc.vector.tensor_reduce(ys[:, T:T + 1], eqf[:], axis=AX, op=MIN)

            # ---- batched backpointer pass on 128 partitions ----
            NS = 128 // BL                    # 16 t-slices
            TSL = T // NS                     # 32 steps per slice
            wp = spool.tile([128, TSL * L9], f32)
            rp = spool.tile([128, TSL * L9], f32)
            vtp = spool.tile([128, TSL * 81], f32)
            eqp = spool.tile([128, TSL * 81], f32)
            ptp = spool.tile([128, TSL * L9], f32)
            ptrh = spool.tile([BL, T * L9], f32)
            dmaeng = [nc.sync, nc.scalar]
            for s in range(NS):
                dmaeng[s % 2].dma_start(
                    wp[s * BL:(s + 1) * BL, :],
                    whist[:, s * TSL * L9:(s + 1) * TSL * L9])
                dmaeng[(s + 1) % 2].dma_start(
                    rp[s * BL:(s + 1) * BL, :],
                    rhist[:, s * TSL * L9:(s + 1) * TSL * L9])
            wpv = _insert_bcast(
                wp[:].rearrange("q (t p) -> q t p", p=L9), 2, L9)
            trv = _insert_bcast(
                trflat[:].rearrange("q (n p) -> q n p", n=L9), 1, TSL)
            iov = _insert_bcast(
                iota81[:].rearrange("q (n p) -> q n p", n=L9), 1, TSL)
            vtpv = vtp[:].rearrange("q (t n p) -> q t n p", n=L9, p=L9)
            eqpv = eqp[:].rearrange("q (t n p) -> q t n p", n=L9, p=L9)
            rpv = rp[:].rearrange("q (t n) -> q t n", n=L9) \
                       .broadcast_to((128, TSL, L9, L9))
            nc.vector.tensor_tensor(vtpv, wpv, trv, op=ADD)
            nc.vector.tensor_tensor(eqpv, vtpv, rpv, op=EQ)
            nc.vector.tensor_tensor(vtpv, eqpv, iov, op=MULT)
            nc.vector.tensor_reduce(
                ptp[:].rearrange("q (t n) -> q t n", n=L9),
                vtpv, axis=AX, op=MIN)
            for s in range(NS):
                dmaeng[s % 2].dma_start(
                    ptrh[:, s * TSL * L9:(s + 1) * TSL * L9],
                    ptp[s * BL:(s + 1) * BL, :])

            # ---- pointer-doubling backtrack ----
            # Compose disjoint pairs of pointer tables (exact integer maps):
            #   C1[tau] = J0[2tau] o J0[2tau+1]   (maps idx_{2tau+2} -> idx_{2tau})
            #   C2[u]   = C1[2u]  o C1[2u+1]      (maps idx_{4u+4} -> idx_{4u})
            #   C3[v]   = C2[2v]  o C2[2v+1]      (maps idx_{8v+8} -> idx_{8v})
            # then run a 63-step chain on C3 and recover skipped steps with
            # batched one-hot gathers.
            def compose(dst, srcT, npairs):
                """dst[q, pair, j] = srcT[q, 2*pair, iv(srcT[q, 2*pair+1, j])]
                in pass layout; srcT has 2*npairs tables of 9 per partition."""
                eqv = bass.AP(eqbufP[:].tensor, eqbufP[:].offset,
                              [list(eqbufP[:].ap[0]), [81, npairs],
                               [L9, L9], [1, L9]])
                odd = bass.AP(srcT.tensor, srcT.offset + L9,
                              [list(srcT.ap[0]), [2 * L9, npairs],
                               [1, L9], [0, L9]])
                iov_ = bass.AP(iota81[:].tensor, iota81[:].offset,
                               [list(iota81[:].ap[0]), [0, npairs],
                                [L9, L9], [1, L9]])
                nc.vector.tensor_tensor(eqv, odd, iov_, op=EQ)
                even = bass.AP(srcT.tensor, srcT.offset,
                               [list(srcT.ap[0]), [2 * L9, npairs],
                                [0, L9], [1, L9]])
                selv = bass.AP(selbufP[:].tensor, selbufP[:].offset,
                               [list(selbufP[:].ap[0]), [81, npairs],
                                [L9, L9], [1, L9]])
                nc.vector.tensor_tensor(selv, eqv, even, op=MULT)
                nc.vector.tensor_reduce(
                    bass.AP(dst.tensor, dst.offset,
                            [list(dst.ap[0]), [L9, npairs], [1, L9]]),
                    selv, axis=AX, op=ADD)

            eqbufP = spool.tile([128, 16 * 81], f32)
            selbufP = spool.tile([128, 16 * 81], f32)
            c1p = spool.tile([128, 16 * L9], f32)
            c2p = spool.tile([128, 8 * L9], f32)
            c3p = spool.tile([128, 4 * L9], f32)
            compose(c1p[:], ptp[:], 16)
            compose(c2p[:], c1p[:], 8)
            compose(c3p[:], c2p[:], 4)

            # relayout compositions to batch-major
            c1h = spool.tile([BL, 256 * L9], f32)
            c2h = spool.tile([BL, 128 * L9], f32)
            c3h = spool.tile([BL, 64 * L9], f32)
            for s in range(NS):
                for ei, (ch, cp, w) in enumerate(
                        ((c1h, c1p, 16 * L9), (c2h, c2p, 8 * L9),
                         (c3h, c3p, 4 * L9))):
                    dmaeng[(s + ei) % 2].dma_start(
                        ch[:, s * w:(s + 1) * w],
                        cp[s * BL:(s + 1) * BL, :])

            # 63-step chain on C3: idx_{8v} = C3[v][idx_{8v+8}]
            oh = spool.tile([BL, L9], f32)
            for v in range(64 - 1, 0, -1):
                nc.vector.scalar_tensor_tensor(
                    oh[:], iota9[:], ys[:, 8 * v + 8:8 * v + 9],
                    c3h[:, v * L9:(v + 1) * L9],
                    op0=EQ, op1=MULT, accum_out=ys[:, 8 * v:8 * v + 1])

            # batched recoveries, coarsest to finest
            eqb = spool.tile([BL, 256 * L9], f32)
            selb = spool.tile([BL, 256 * L9], f32)

            def recover(table, cnt, in_off, in_step, tab_off, tab_step,
                        out_off, out_step):
                """ys[out] = table_odd[ys[in]] for cnt positions at once."""
                eqv = bass.AP(eqb[:].tensor, eqb[:].offset,
                              [list(eqb[:].ap[0]), [L9, cnt], [1, L9]])
                io_b = bass.AP(iota9[:].tensor, iota9[:].offset,
                               [list(iota9[:].ap[0]), [0, cnt], [1, L9]])
                ys_in = bass.AP(ys[:].tensor, ys[:].offset + in_off,
                                [list(ys[:].ap[0]), [in_step, cnt], [0, L9]])
                nc.vector.tensor_tensor(eqv, io_b, ys_in, op=EQ)
                selv = bass.AP(selb[:].tensor, selb[:].offset,
                               [list(selb[:].ap[0]), [L9, cnt], [1, L9]])
                tab = bass.AP(table.tensor, table.offset + tab_off,
                              [list(table.ap[0]), [tab_step, cnt], [1, L9]])
                nc.vector.tensor_tensor(selv, eqv, tab, op=MULT)
                ys_out = bass.AP(ys[:].tensor, ys[:].offset + out_off,
                                 [list(ys[:].ap[0]), [out_step, cnt]])
                nc.vector.tensor_reduce(
                    ys_out,
                    bass.AP(selb[:].tensor, selb[:].offset,
                            [list(selb[:].ap[0]), [L9, cnt], [1, L9]]),
                    axis=AX, op=ADD)

            # level-2: idx_{8v+4} = C2[2v+1][idx_{8v+8}], v=0..63
            recover(c2h[:], 64, 8, 8, L9, 2 * L9, 4, 8)
            # level-1: idx_{4u+2} = C1[2u+1][idx_{4u+4}], u=0..127
            recover(c1h[:], 128, 4, 4, L9, 2 * L9, 2, 4)
            # level-0: idx_{2tau+1} = J0[2tau+1][idx_{2tau+2}], tau=0..255
            recover(ptrh[:], 256, 2, 2, L9, 2 * L9, 1, 2)

            # ---- assemble preds ----
            predsF = spool.tile([BL, T], f32)
            predsI = spool.tile([BL, T], i32)
            nc.vector.tensor_scalar_add(predsF[:], ys[:, 1:T + 1], SHIFT)
            nc.vector.tensor_copy(predsI[:], predsF[:])
            nc.sync.dma_start(preds_d[:], predsI[:])

    nc.compile()
    return nc


def _host_inputs(inputs, W1, b1, W2, b2, transition):
    """Build the per-core in_maps."""
    W1T = np.ascontiguousarray(W1.T)                      # [768, 384]
    W2T = np.ascontiguousarray(W2.T)                      # [384, 9]
    w1t = np.ascontiguousarray(
        W1T.reshape(KT1, 128, HID).transpose(1, 0, 2).reshape(128, KT1 * HID))
    w2t = np.ascontiguousarray(
        W2T.reshape(MT1, 128, L9).transpose(1, 0, 2).reshape(128, MT1 * L9))
    b1c = np.ascontiguousarray(b1.reshape(MT1, 128).T)
    b2c = np.ascontiguousarray(b2.reshape(L9, 1))
    Tr9 = transition[:L9, :L9].astype(np.float32)
    trflat = np.tile(Tr9.reshape(1, 81), (128, 1))
    iota_p = np.arange(L9, dtype=np.float32) - SHIFT
    iota81 = np.tile(np.tile(iota_p, L9).reshape(1, 81), (128, 1))
    trcol9 = np.tile(transition[:L9, L9 + 0].reshape(1, L9), (BL, 1))
    trow10 = np.tile(transition[L9 + 1, :L9].reshape(1, L9), (BL, 1))
    iota9 = np.tile(iota_p.reshape(1, L9), (BL, 1))
    ident = np.eye(L9, dtype=np.float32)
    b2row = np.tile(b2.reshape(1, L9), (1, 4)).astype(np.float32)
    ones1 = np.ones((1, 128), np.float32)

    in_maps = []
    for c in range(NCORES):
        # token order (t, b): tok = t*BL + b  -> chunk = contiguous t range
        Xc = inputs[c * BL:(c + 1) * BL].transpose(1, 0, 2).reshape(BL * T, D)
        xt = np.ascontiguousarray(Xc.T)                  # [768, BL*T]
        in_maps.append({
            "xt": xt, "w1t": w1t, "w2t": w2t, "b1c": b1c, "b2c": b2c,
            "trflat": trflat, "iota81": iota81, "trcol9": trcol9,
            "trow10": trow10, "iota9": iota9, "ident": ident,
            "b2row": b2row, "ones1": ones1,
        })
    return in_maps


def kernel(inputs, labels_mask, W1, b1, W2, b2, transition):
    inputs = np.asarray(inputs, np.float32)
    labels_mask = np.asarray(labels_mask, np.int32)
    W1 = np.asarray(W1, np.float32)
    b1 = np.asarray(b1, np.float32)
    W2 = np.asarray(W2, np.float32)
    b2 = np.asarray(b2, np.float32)
    transition = np.asarray(transition, np.float32)

    if not np.all(labels_mask == 1):
        return _np_fallback(inputs, labels_mask, W1, b1, W2, b2, transition)

    from concourse import bass_utils

    if "nc" not in _cache:
        _cache["nc"] = _build_program()
    nc = _cache["nc"]

    in_maps = _host_inputs(inputs, W1, b1, W2, b2, transition)
    res = bass_utils.run_bass_kernel_spmd(
        nc, in_maps, core_ids=list(range(NCORES)))
    _cache["last_res"] = res
    preds = np.concatenate(
        [np.asarray(res.results[c]["preds"]) for c in range(NCORES)], axis=0)
    return preds.astype(np.int32)


if __name__ == "__main__":
    import reference
    ins = reference.setup_inputs()
    ins = {k: np.asarray(v) for k, v in ins.items()}
    out = kernel(**ins)
    print("kernel out", out.shape, out.dtype)
